# revision 5
# baseline (speedup 1.0000x reference)
"""DistMult edge scoring on 8 TRN2 cores — one-hot-matmul src gather.

score[e] = sum_d node_emb[src[e],d] * rel_emb[e,d] * node_emb[dst[e],d]

Structure (per core, src-range sharding: core c owns src in
[12500c, 12500(c+1))):
  - Edges grouped by src into NG=104 groups (consecutive node runs,
    <=128 nodes and <=TPB*128 edges each); each group = TPB=10 tiles of
    128 edges, dst-sorted within the group, pads spread evenly.
  - h (src row, full f32): PE one-hot gather. Stationary = fp8 one-hot
    [node, edge] (host stream); moving = resident bf16 hi/lo src-slice
    rows; two matmuls accumulate hi+lo into PSUM = exact-ish f32 h.
  - t (dst row): classic dma_gather of f32 rows (256B descriptors) with
    per-tile quantile bases so signed-int16 offsets always fit.
  - ACT drains PSUM->SBUF; DVE does u=t*rel, prod=u*h, reduce.
"""

import numpy as np

import concourse.bacc as bacc
import concourse.bass as bass
import concourse.mybir as mybir
from concourse import library_config
from concourse.bass_utils import run_bass_kernel_spmd

N_NODES = 100000
DIM = 64
N_EDGES = 1000000
N_CORES = 8
SLICE = N_NODES // N_CORES     # 12500 src nodes per core

NG = 104                       # groups per core
TPB = 10                       # tiles per group
SC = 8                         # groups per superchunk
NSC = NG // SC                 # 13 superchunks
TILE = 128
GEDGE = TPB * TILE             # 1280 edge slots per group
SLOTS = NG * GEDGE             # 133120 slots per core
BASES = [max(0, k * (N_NODES // TPB) - 8000) for k in range(TPB)]

F32 = mybir.dt.float32
BF16 = mybir.dt.bfloat16
FP8 = mybir.dt.float8e4
I16 = mybir.dt.int16

_cache = {}


def _build_program():
    if "nc" in _cache:
        return _cache["nc"]
    nc = bacc.Bacc(
        "TRN2",
        target_bir_lowering=False,
        debug=False,
        enable_asserts=False,
        num_devices=N_CORES,
        num_swdge_queues=4,
    )
    tbl = nc.dram_tensor("tbl", [N_NODES, DIM], F32, kind="ExternalInput")
    hsl_h = nc.dram_tensor("hsl", [128, NG, 2, DIM], BF16, kind="ExternalInput")
    oh_h = nc.dram_tensor("oh", [NSC, 128, SC, TPB, TILE], FP8, kind="ExternalInput")
    idx_h = nc.dram_tensor("idx", [NSC, 128, TPB, SC * TILE // 16], I16,
                           kind="ExternalInput")
    rel_h = nc.dram_tensor("rel", [NSC, 128, TPB, SC, DIM], F32,
                           kind="ExternalInput")
    out_h = nc.dram_tensor("score", [NSC, 128, TPB, SC], F32,
                           kind="ExternalOutput")

    from contextlib import ExitStack

    es = ExitStack()
    with es:
        block = es.enter_context(nc.Block())
        hsl = es.enter_context(nc.sbuf_tensor("hslb", [128, NG, 2, DIM], BF16))
        ohb = es.enter_context(
            nc.sbuf_tensor("ohb", [128, 2, SC, TPB, TILE], FP8))
        idxb = es.enter_context(
            nc.sbuf_tensor("idxb", [128, 2, TPB, SC * TILE // 16], I16))
        relb = es.enter_context(
            nc.sbuf_tensor("relb", [128, 2, TPB, SC, DIM], F32))
        gtb = es.enter_context(
            nc.sbuf_tensor("gtb", [128, 2, TPB, SC, DIM], F32))
        stb = es.enter_context(
            nc.sbuf_tensor("stb", [128, 2, TPB, SC, DIM], F32))
        scob = es.enter_context(nc.sbuf_tensor("scob", [128, 2, TPB, SC], F32))
        ps = es.enter_context(nc.psum_tensor("ps", [128, 2, 16, DIM], F32))

        s_hsl = es.enter_context(nc.semaphore("s_hsl"))
        s_oh = es.enter_context(nc.semaphore("s_oh"))
        s_idx = es.enter_context(nc.semaphore("s_idx"))
        s_rel = es.enter_context(nc.semaphore("s_rel"))
        s_mm = es.enter_context(nc.semaphore("s_mm"))     # per-group 2*TPB mms
        s_dr = es.enter_context(nc.semaphore("s_dr"))     # ACT drains (1/group)
        s_v = es.enter_context(nc.semaphore("s_v"))       # DVE ops (3/sc)
        s_out = es.enter_context(nc.semaphore("s_out"))
        sk = tuple(
            tuple(
                es.enter_context(nc.semaphore(f"sk{k}{p}")) for p in range(2)
            )
            for k in range(TPB)
        )

        MM = 2 * TPB  # matmuls per group

        @block.sync
        def _(sp: bass.BassEngine):
            sp.dma_start(out=hsl[:], in_=hsl_h[:]).then_inc(s_hsl, 16)
            for s in range(NSC):
                b = s % 2
                if s >= 1:
                    # order completions: count N implies chunks 0..N-1 resident
                    sp.wait_ge(s_idx, 16 * s)
                    sp.wait_ge(s_oh, 16 * s)
                    sp.wait_ge(s_rel, 16 * s)
                if s >= 2:
                    # oh/idx/rel buffer b free when superchunk s-2 consumed:
                    # oh consumed by PE (all mms of sc s-2 done)
                    sp.wait_ge(s_mm, MM * SC * (s - 1))
                    # idx consumed by gathers of sc s-2 (same parity as s)
                    for k in range(TPB):
                        sp.wait_ge(sk[k][s % 2], 16 * (s // 2))
                    # rel consumed by DVE of sc s-2
                    sp.wait_ge(s_v, 3 * (s - 1))
                sp.dma_start(out=idxb[:, b], in_=idx_h[s]).then_inc(s_idx, 16)
                sp.dma_start(out=ohb[:, b], in_=oh_h[s]).then_inc(s_oh, 16)
                sp.dma_start(out=relb[:, b], in_=rel_h[s]).then_inc(s_rel, 16)

        @block.gpsimd
        def _(gp: bass.BassGpSimd):
            gp.load_library(library_config.mlp)
            for s in range(NSC):
                b = s % 2
                gp.wait_ge(s_idx, 16 * (s + 1))
                if s >= 2:
                    # gt buffer b free when DVE consumed sc s-2 (reduce done)
                    gp.wait_ge(s_v, 3 * (s - 1))
                for k in range(TPB):
                    gp.dma_gather(
                        gtb[:, b, k],
                        tbl[BASES[k]:],
                        idxb[:, b, k],
                        SC * TILE,
                        SC * TILE,
                        DIM,
                        elem_step=DIM,
                        single_packet=True,
                        queue_num=(k + 2 * s) % 4,
                    ).then_inc(sk[k][s % 2], 16)

        @block.tensor
        def _(pe: bass.BassTensorEngine):
            pe.wait_ge(s_hsl, 16)
            for s in range(NSC):
                b = s % 2
                pe.wait_ge(s_oh, 16 * (s + 1))
                for g in range(SC):
                    gg = s * SC + g
                    pb = gg % 2
                    if gg >= 2:
                        # psum buffer pb free when group gg-2 drained
                        pe.wait_ge(s_dr, gg - 1)
                    for k in range(TPB):
                        oh_ap = ohb[:, b, g, k]
                        pe.matmul(
                            ps[:, pb, k],
                            oh_ap,
                            hsl[:, gg, 0],
                            start=True,
                            stop=False,
                        ).then_inc(s_mm, 1)
                        pe.matmul(
                            ps[:, pb, k],
                            oh_ap,
                            hsl[:, gg, 1],
                            start=False,
                            stop=True,
                        ).then_inc(s_mm, 1)

        @block.scalar
        def _(sc_e: bass.BassScalarEngine):
            for s in range(NSC):
                b = s % 2
                for g in range(SC):
                    gg = s * SC + g
                    pb = gg % 2
                    sc_e.wait_ge(s_mm, MM * (gg + 1))
                    if s >= 2:
                        # st buffer b free when DVE consumed sc s-2
                        sc_e.wait_ge(s_v, 3 * (s - 1))
                    # drain psum group gg -> st[:, b, :, g, :]
                    sc_e.copy(
                        out=stb[:, b, :, g], in_=ps[:, pb, :TPB]
                    ).then_inc(s_dr, 1)
                # out-store for sc s-2 after DVE finished it
                if s >= 2:
                    sc_e.wait_ge(s_v, 3 * (s - 1))
                    sc_e.wait_ge(s_out, 16 * (s - 2))
                    sc_e.dma_start(
                        out=out_h[s - 2], in_=scob[:, s % 2]
                    ).then_inc(s_out, 16)
            for s in (NSC - 2, NSC - 1):
                sc_e.wait_ge(s_v, 3 * (s + 1))
                sc_e.wait_ge(s_out, 16 * s)
                sc_e.dma_start(
                    out=out_h[s], in_=scob[:, s % 2]
                ).then_inc(s_out, 16)
            sc_e.wait_ge(s_out, 16 * NSC)

        @block.vector
        def _(v: bass.BassEngine):
            mult = mybir.AluOpType.mult
            for s in range(NSC):
                b = s % 2
                v.wait_ge(s_rel, 16 * (s + 1))
                # all gathers of this superchunk landed
                for k in range(TPB):
                    v.wait_ge(sk[k][s % 2], 16 * (s // 2 + 1))
                # all drains of this superchunk done
                v.wait_ge(s_dr, SC * (s + 1))
                if s >= 2:
                    # scob buffer free when out-store of sc s-2 done
                    v.wait_ge(s_out, 16 * (s - 1))
                # u = t * rel  (overwrite gt)
                v.tensor_tensor(
                    out=gtb[:, b], in0=gtb[:, b], in1=relb[:, b], op=mult
                ).then_inc(s_v, 1)
                # prod = u * h (overwrite gt)
                v.wait_ge(s_v, 3 * s + 1)
                v.tensor_tensor(
                    out=gtb[:, b], in0=gtb[:, b], in1=stb[:, b], op=mult
                ).then_inc(s_v, 1)
                # score = reduce_d prod
                v.wait_ge(s_v, 3 * s + 2)
                v.tensor_reduce(
                    out=scob[:, b],
                    in_=gtb[:, b],
                    axis=mybir.AxisListType.X,
                    op=mybir.AluOpType.add,
                ).then_inc(s_v, 1)

    nc.compile()
    _cache["nc"] = nc
    return nc


def _pack_groups(src_c, c):
    """Pack the core's src-slice nodes into NG consecutive runs with
    <=128 nodes and <=GEDGE edges each. Returns group start/end node ids
    and per-edge group assignment implicitly via node->group."""
    lo = c * SLICE
    deg = np.bincount(src_c - lo, minlength=SLICE)

    def greedy(cap):
        n2g = np.empty(SLICE, np.int32)
        g = 0
        cnt = 0
        nn = 0
        for n in range(SLICE):
            if nn >= 128 or (cnt + deg[n] > cap and cnt > 0):
                g += 1
                cnt = 0
                nn = 0
            n2g[n] = g
            cnt += deg[n]
            nn += 1
        return g + 1, n2g

    lo_cap, hi_cap = int(deg.max()), GEDGE
    # smallest cap gives most groups; find cap with exactly NG groups
    best = None
    lo_c, hi_c = 64, GEDGE
    while lo_c <= hi_c:
        mid = (lo_c + hi_c) // 2
        ngr, n2g = greedy(mid)
        if ngr > NG:
            lo_c = mid + 1
        else:
            best = (ngr, n2g, mid)
            hi_c = mid - 1
    assert best is not None, "cannot pack into NG groups"
    ngr, n2g, cap = best
    assert ngr <= NG and cap <= GEDGE, (ngr, cap)
    # per-group edge counts
    gcnt = np.bincount(n2g[src_c - lo], minlength=NG)
    assert gcnt.max() <= GEDGE, gcnt.max()
    return n2g


def _shard_inputs(node_emb, rel_emb, src, dst):
    import ml_dtypes

    node_emb = np.asarray(node_emb, dtype=np.float32)
    rel_emb = np.asarray(rel_emb, dtype=np.float32)
    src = np.asarray(src, dtype=np.int64)
    dst = np.asarray(dst, dtype=np.int64)

    hi_full = node_emb.astype(ml_dtypes.bfloat16)
    lo_full = (node_emb - hi_full.astype(np.float32)).astype(ml_dtypes.bfloat16)

    core_of = (src // SLICE).astype(np.int32)
    in_maps = []
    infos = []
    for c in range(N_CORES):
        eids = np.nonzero(core_of == c)[0]
        src_c = src[eids]
        dst_c = dst[eids]
        lo = c * SLICE
        n2g = _pack_groups(src_c, c)
        egrp = n2g[src_c - lo]

        # slot assignment: within group sort by dst, spread across TPB
        # tiles evenly, pad each tile to 128
        slot_of = np.full(SLOTS, -1, np.int64)  # slot -> edge id (global)
        order = np.lexsort((dst_c, egrp))
        gcnt = np.bincount(egrp, minlength=NG)
        pos = 0
        for g in range(NG):
            ge = order[pos:pos + gcnt[g]]
            pos += gcnt[g]
            eg = gcnt[g]
            if eg >= 640:
                bounds = np.ceil(
                    np.arange(TPB + 1) * eg / TPB
                ).astype(np.int64)
                tiles = [ge[bounds[k]:bounds[k + 1]] for k in range(TPB)]
            else:
                tk = np.minimum(dst_c[ge] // (N_NODES // TPB), TPB - 1)
                tiles = [ge[tk == k] for k in range(TPB)]
            for k in range(TPB):
                te = tiles[k]
                assert len(te) <= TILE, (g, k, len(te))
                base_slot = (g * TPB + k) * TILE
                slot_of[base_slot:base_slot + len(te)] = eids[te]

        filled = slot_of >= 0
        s_src = np.where(filled, src[np.maximum(slot_of, 0)], -1)
        s_dst = np.where(filled, dst[np.maximum(slot_of, 0)], -1)

        # one-hot fp8 [NG, TPB, 128n, 128e]
        nloc = np.where(filled, s_src - lo, 0)
        # group start node (local): first node with n2g==g
        gstart = np.zeros(NG, np.int64)
        idxs = np.nonzero(np.diff(np.concatenate([[-1], n2g])))[0]
        for gi, st in zip(n2g[idxs], idxs):
            gstart[gi] = st
        srow = nloc.reshape(NG, TPB, TILE) - gstart[:, None, None]
        oh = (
            (srow[:, :, None, :] == np.arange(128)[None, None, :, None])
            & filled.reshape(NG, TPB, TILE)[:, :, None, :]
        )
        assert ((srow >= 0) & (srow < 128) | ~filled.reshape(NG, TPB, TILE)).all()
        # -> [NSC, 128, SC, TPB, TILE]
        oh8 = (
            oh.reshape(NSC, SC, TPB, 128, TILE)
            .transpose(0, 3, 1, 2, 4)
            .astype(np.float32)
            .astype(ml_dtypes.float8_e4m3)
        )

        # hsl [128, NG, 2, DIM] bf16: partition p of group g = node gstart+p
        nidx = np.minimum(gstart[None, :] + np.arange(128)[:, None], SLICE - 1)
        valid = (gstart[None, :] + np.arange(128)[:, None]) < SLICE
        hsl = np.zeros((128, NG, 2, DIM), ml_dtypes.bfloat16)
        hsl[:, :, 0] = np.where(valid[:, :, None], hi_full[lo + nidx], 0)
        hsl[:, :, 1] = np.where(valid[:, :, None], lo_full[lo + nidx], 0)

        # gather idx int16 offsets [NSC, TPB, SC*TILE] wrapped
        # slot j within (s, k): j = g_local*128 + p -> edge slot
        # (g = s*SC+g_local, k, p)
        sdst = s_dst.reshape(NG, TPB, TILE)
        offs = np.where(
            sdst >= 0, sdst - np.array(BASES)[None, :, None], 0
        ).astype(np.int64)
        assert (offs >= 0).all() and (offs <= 32767).all(), (
            offs.min(), offs.max())
        # [NSC, SC, TPB, TILE] -> [NSC, TPB, SC, TILE] -> [NSC, TPB, SC*TILE]
        offs = (
            offs.reshape(NSC, SC, TPB, TILE)
            .transpose(0, 2, 1, 3)
            .reshape(NSC, TPB, SC * TILE)
            .astype(np.int16)
        )
        # wrap 16 + replicate to 128 partitions
        wr = offs.reshape(NSC, TPB, SC * TILE // 16, 16).swapaxes(2, 3)
        idx16 = np.broadcast_to(
            wr[:, None, :, :, :], (NSC, 8, TPB, 16, SC * TILE // 16)
        ).transpose(0, 1, 3, 2, 4).reshape(NSC, 128, TPB, SC * TILE // 16)

        # rel [NSC, 128, TPB, SC, DIM]: edge (g=s*SC+gl, k, p) -> [s, p, k, gl]
        rel_slot = np.where(
            filled[:, None], rel_emb[np.maximum(slot_of, 0)], 0.0
        ).astype(np.float32)
        rel_t = np.ascontiguousarray(
            rel_slot.reshape(NSC, SC, TPB, TILE, DIM).transpose(0, 3, 2, 1, 4)
        )

        in_maps.append(
            {
                "tbl": node_emb,
                "hsl": hsl,
                "oh": np.ascontiguousarray(oh8),
                "idx": np.ascontiguousarray(idx16),
                "rel": rel_t,
            }
        )
        infos.append(slot_of)
    return in_maps, infos


def run_on_hw(node_emb, rel_emb, src, dst, **spmd_kwargs):
    nc = _build_program()
    in_maps, infos = _shard_inputs(node_emb, rel_emb, src, dst)
    res = run_bass_kernel_spmd(nc, in_maps, list(range(N_CORES)), **spmd_kwargs)
    out = np.empty(N_EDGES, np.float32)
    for c in range(N_CORES):
        # score [NSC, 128, TPB, SC] -> slot (g=s*SC+gl, k, p)
        sc_arr = np.asarray(res.results[c]["score"])
        slot_scores = sc_arr.transpose(0, 3, 2, 1).reshape(SLOTS)
        # slot layout: (g * TPB + k) * TILE + p where g = s*SC+gl
        slot_of = infos[c]
        m = slot_of >= 0
        out[slot_of[m]] = slot_scores[m]
    return out, res


def kernel(node_emb, rel_emb, src, dst):
    scores, _ = run_on_hw(node_emb, rel_emb, src, dst)
    return scores
